# revision 29
# baseline (speedup 1.0000x reference)
"""DPLR SSM block kernel for Trainium2, 8 NeuronCores.

Math:  out = h @ (diag(a_diag) + p q^T).T + x @ b_mat          (B=64, H=8192, R=4)
           = h * a_diag  +  (h @ q) @ p^T  +  x @ b_mat

The dense (H,H) DPLR matrix is never materialized.  The memory-bound part is
streaming b_mat (256 MB fp32-worth of data).  Sharding: b_mat columns (= output
features) are split 8 ways; each core computes out[:, c*1024:(c+1)*1024].
x / q are replicated; host pre-permutes everything into the k-on-partitions
chunk layout the tensor engine wants, so no on-device transposes are needed.

fp32 matmul streams at 4 cycles/row on the PE, which would make the tensor
engine the critical path (~110us/core) over the DMA stream (~100us/core).
Instead x and b are carried as bf16 (hi, lo) pairs -- b ~= bh + bl with
bh = bf16(b), bl = bf16(b - bh) -- and the product uses three full-rate bf16
matmuls accumulating in fp32 PSUM:
    x @ b ~= xh@bh + xl@bh + xh@bl     (error ~1e-5, fp32-grade)
dropping only the xl@bl term (~2^-18 relative).  HBM traffic is unchanged
(2+2 bytes/element), but PE time drops to ~85us, restoring the DMA roofline.

Per core c (j0 = c*1024):
  hqT (4, 64)       = sum_ko  q[ko]^T(4x128) . hT[ko](128x64)          [PE fp32]
  ps  (64, 1024)    = 3-pass split-bf16 x @ b_slice                    [PE bf16]
                    + hqT^T(64x4) . pT(4x1024)                         [PE fp32]
  out (64, 1024)    = h_slice * a_slice  +  ps                         [DVE]
"""

import ml_dtypes
import numpy as np

import concourse.bass as bass
import concourse.mybir as mybir
from concourse import bacc
from concourse.bass_utils import run_bass_kernel_spmd
from concourse.tile import TileContext

H = 8192
R = 4
B = 64
NCORES = 8
JS = H // NCORES  # 1024 output columns per core
P = 128
KO = H // P  # 64 k-chunks
KT = 4  # k-chunks per DMA tile (tile = 128 x 4 x 2 x 1024 bf16 = 2 MB)
NT = KO // KT  # 16 b-mat DMA tiles per core

F32 = mybir.dt.float32
BF16 = mybir.dt.bfloat16
BF = ml_dtypes.bfloat16


def _build_nc(
    tiles: list[int] | None = None,
    bufs: int = 6,
    hq_tiles: tuple[int, int] = (4, 8),
    rank4_tile: int = 9,
    loop_n: int | None = None,
    aux_in_loop: bool = False,
) -> bass.Bass:
    nc = bacc.Bacc("TRN2", target_bir_lowering=False, debug=False, num_devices=NCORES)

    xh = nc.dram_tensor("xh", (P, KO, B), BF16, kind="ExternalInput")
    xl = nc.dram_tensor("xl", (P, KO, B), BF16, kind="ExternalInput")
    ht = nc.dram_tensor("ht", (P, KO, B), F32, kind="ExternalInput")
    qk = nc.dram_tensor("qk", (P, KO, R), F32, kind="ExternalInput")
    pt = nc.dram_tensor("pt", (R, JS), F32, kind="ExternalInput")
    bm = nc.dram_tensor("bm", (P, KO, 2, JS), BF16, kind="ExternalInput")
    hs = nc.dram_tensor("hs", (B, JS), F32, kind="ExternalInput")
    ab = nc.dram_tensor("ab", (1, JS), F32, kind="ExternalInput")
    o = nc.dram_tensor("o", (B, JS), F32, kind="ExternalOutput")

    # b-tile sizes in k-chunks.  Tapered at both ends: small first tiles so
    # the PE can start as soon as possible, small last tiles so that after
    # the final DMA byte lands only one small tile's matmuls remain.
    TILES = tiles if tiles is not None else [1, 1, 2] + [4] * 14 + [2, 1, 1]
    assert sum(TILES) == KO
    MAXKT = max(TILES)

    with TileContext(nc) as tc:
        with (
            tc.tile_pool(name="persist", bufs=1) as persist,
            tc.tile_pool(name="bpool", bufs=bufs) as bpool,
            tc.tile_pool(name="psum", bufs=1, space="PSUM") as psum_pool,
        ):
            # Aux loads on the scalar HWDGE ring so the b stream on nc.sync
            # isn't delayed.  qk/ht-chunks/xh/xl are ordered so the PE's hq
            # matmul groups and first main tiles can start as early as
            # possible; hq groups are interleaved between the first main
            # tiles to fill the PE while the DMA stream warms up.
            xh_sb = persist.tile([P, KO, B], BF16)
            xl_sb = persist.tile([P, KO, B], BF16)
            qk_sb = persist.tile([P, KO, R], F32)
            ht_sb = persist.tile([P, KO, B], F32)
            pt_sb = persist.tile([R, JS], F32)
            hs_sb = persist.tile([B, JS], F32)
            a1_sb = persist.tile([1, JS], F32)
            ab_sb = persist.tile([B, JS], F32)

            def emit_aux():
                nc.scalar.dma_start(out=xh_sb[:], in_=xh[:, :, :])
                nc.scalar.dma_start(out=xl_sb[:], in_=xl[:, :, :])
                nc.scalar.dma_start(out=qk_sb[:], in_=qk[:, :, :])
                HT_CH = KO // 4
                for hc in range(4):
                    ksl = slice(hc * HT_CH, (hc + 1) * HT_CH)
                    nc.scalar.dma_start(out=ht_sb[:, ksl], in_=ht[:, ksl, :])
                nc.scalar.dma_start(out=pt_sb[:], in_=pt[:, :])
                nc.scalar.dma_start(out=hs_sb[:], in_=hs[:, :])
                # a_diag slice arrives as one row; broadcast to all 64 batch
                # partitions on the (otherwise idle) GPSIMD engine.
                nc.scalar.dma_start(out=a1_sb[:], in_=ab[:, :])
                nc.gpsimd.partition_broadcast(ab_sb[:], a1_sb[:])

            out_sb = persist.tile([B, JS], F32)
            hqt_sb = persist.tile([R, B], F32)

            import contextlib

            loop_ctx = (
                tc.For_i(0, loop_n, 1, hint_engines=(mybir.EngineType.PE,))
                if loop_n
                else contextlib.nullcontext()
            )
            if not (loop_n and aux_in_loop):
                emit_aux()
            with loop_ctx:
                if loop_n and aux_in_loop:
                    emit_aux()
                _emit_body(
                    nc, tc, TILES, MAXKT, bpool, psum_pool, persist,
                    qk_sb, ht_sb, xh_sb, xl_sb, pt_sb, hs_sb, ab_sb,
                    out_sb, hqt_sb, bm, o, hq_tiles, rank4_tile,
                )

    nc.finalize()
    return nc


def _emit_body(
    nc, tc, TILES, MAXKT, bpool, psum_pool, persist,
    qk_sb, ht_sb, xh_sb, xl_sb, pt_sb, hs_sb, ab_sb,
    out_sb, hqt_sb, bm, o, hq_tiles, rank4_tile,
):
            ps0 = psum_pool.tile([B, 512], F32)
            ps1 = psum_pool.tile([B, 512], F32)
            ps2 = psum_pool.tile([B, 512], F32)
            ps3 = psum_pool.tile([B, 512], F32)
            pshq = psum_pool.tile([R, B], F32)

            # Diagonal term early (off the critical tail).
            nc.vector.tensor_mul(out=out_sb[:], in0=hs_sb[:], in1=ab_sb[:])

            def hq_group(g):
                # hqT = q^T @ h^T for k-chunks [16g, 16g+16) (fp32).
                for ko in range(16 * g, 16 * (g + 1)):
                    nc.tensor.matmul(
                        pshq[:],
                        qk_sb[:, ko],
                        ht_sb[:, ko],
                        start=(ko == 0),
                        stop=(ko == KO - 1),
                    )

            # Main stream: x @ b_slice via 3-pass split-bf16.
            ko = 0
            for t, kt in enumerate(TILES):
                if hq_tiles[0] <= t < hq_tiles[1]:
                    g, ng = t - hq_tiles[0], hq_tiles[1] - hq_tiles[0]
                    if ng == 4:
                        hq_group(g)
                    elif ng == 2:
                        hq_group(2 * g)
                        hq_group(2 * g + 1)
                if t == rank4_tile:
                    # Rank-4 term into its own PSUM banks, mid-stream.
                    nc.vector.tensor_copy(out=hqt_sb[:], in_=pshq[:])
                    nc.tensor.matmul(
                        ps2[:], hqt_sb[:], pt_sb[:, 0:512], start=True, stop=True
                    )
                    nc.tensor.matmul(
                        ps3[:], hqt_sb[:], pt_sb[:, 512:JS], start=True, stop=True
                    )
                bfull = bpool.tile([P, MAXKT, 2, JS], BF16, name="btile")
                btile = bfull[:, :kt]
                dma_eng = nc.sync if t % 2 == 0 else nc.scalar
                dma_eng.dma_start(out=btile[:], in_=bm[:, ko : ko + kt])
                for k4 in range(kt):
                    st = ko == 0
                    lst = ko == KO - 1
                    bh = btile[:, k4, 0]
                    bl = btile[:, k4, 1]
                    nc.tensor.matmul(
                        ps0[:], xh_sb[:, ko], bh[:, 0:512], start=st, stop=False
                    )
                    nc.tensor.matmul(
                        ps1[:], xh_sb[:, ko], bh[:, 512:JS], start=st, stop=False
                    )
                    nc.tensor.matmul(
                        ps0[:], xh_sb[:, ko], bl[:, 0:512], start=False, stop=False
                    )
                    nc.tensor.matmul(
                        ps1[:], xh_sb[:, ko], bl[:, 512:JS], start=False, stop=False
                    )
                    # xl-stationary last: xl arrives after xh at startup.
                    nc.tensor.matmul(
                        ps0[:], xl_sb[:, ko], bh[:, 0:512], start=False, stop=lst
                    )
                    nc.tensor.matmul(
                        ps1[:], xl_sb[:, ko], bh[:, 512:JS], start=False, stop=lst
                    )
                    ko += 1

            # Rank-4 folded into out_sb mid-stream (off the critical tail).
            nc.vector.tensor_add(
                out=out_sb[:, 0:512], in0=out_sb[:, 0:512], in1=ps2[:]
            )
            nc.vector.tensor_add(
                out=out_sb[:, 512:JS], in0=out_sb[:, 512:JS], in1=ps3[:]
            )

            # Tail: fold the main accumulators and store.
            nc.vector.tensor_add(
                out=out_sb[:, 0:512], in0=out_sb[:, 0:512], in1=ps0[:]
            )
            nc.sync.dma_start(out=o[:, 0:512], in_=out_sb[:, 0:512])
            nc.vector.tensor_add(
                out=out_sb[:, 512:JS], in0=out_sb[:, 512:JS], in1=ps1[:]
            )
            nc.scalar.dma_start(out=o[:, 512:JS], in_=out_sb[:, 512:JS])


_NC_CACHE = None


def _get_nc() -> bass.Bass:
    global _NC_CACHE
    if _NC_CACHE is None:
        _NC_CACHE = _build_nc()
    return _NC_CACHE


def _split_bf16(a: np.ndarray) -> tuple[np.ndarray, np.ndarray]:
    hi = a.astype(BF)
    lo = (a - hi.astype(np.float32)).astype(BF)
    return hi, lo


def _in_maps(h, x, a_diag, p_vec, q_vec, b_mat):
    # Replicated inputs, pre-permuted to k-on-partitions chunk layout.
    # xt[ki, ko, b] = x[b, ko*128 + ki]
    xt = np.ascontiguousarray(x.reshape(B, KO, P).transpose(2, 1, 0))
    xh, xl = _split_bf16(xt)
    ht = np.ascontiguousarray(h.reshape(B, KO, P).transpose(2, 1, 0))
    # qk[ki, ko, r] = q_vec[ko*128 + ki, r]
    qk = np.ascontiguousarray(q_vec.reshape(KO, P, R).transpose(1, 0, 2))

    # b4[ko, ki, c, j] = b_mat[ko*128 + ki, c*1024 + j]
    b4 = b_mat.reshape(KO, P, NCORES, JS)

    in_maps = []
    for c in range(NCORES):
        j0 = c * JS
        bc = np.ascontiguousarray(b4[:, :, c, :].transpose(1, 0, 2))  # (P, KO, JS)
        bh, bl = _split_bf16(bc)
        bhl = np.ascontiguousarray(np.stack([bh, bl], axis=2))  # (P, KO, 2, JS)
        in_maps.append(
            {
                "xh": xh,
                "xl": xl,
                "ht": ht,
                "qk": qk,
                "pt": np.ascontiguousarray(p_vec[j0 : j0 + JS, :].T),
                "bm": bhl,
                "hs": np.ascontiguousarray(h[:, j0 : j0 + JS]),
                "ab": np.ascontiguousarray(a_diag[j0 : j0 + JS]).reshape(1, JS),
            }
        )
    return in_maps


def kernel(h, x, a_diag, p_vec, q_vec, b_mat) -> np.ndarray:
    h = np.ascontiguousarray(np.asarray(h, dtype=np.float32))
    x = np.ascontiguousarray(np.asarray(x, dtype=np.float32))
    a_diag = np.asarray(a_diag, dtype=np.float32)
    p_vec = np.asarray(p_vec, dtype=np.float32)
    q_vec = np.asarray(q_vec, dtype=np.float32)
    b_mat = np.asarray(b_mat, dtype=np.float32)

    nc = _get_nc()
    res = run_bass_kernel_spmd(
        nc, _in_maps(h, x, a_diag, p_vec, q_vec, b_mat), core_ids=list(range(NCORES))
    )
    return np.concatenate([r["o"] for r in res.results], axis=1)
